# revision 30
# baseline (speedup 1.0000x reference)
"""Causal self-attention (B=2, T=2048, dim=2048, 16 heads, RoPE) on 8 trn2
NeuronCores.

Sharding: core c handles batch b = c//4 and head group g = c%4 (4 heads each,
tensor-parallel over heads). Each core computes QKV projection + RoPE +
causal attention + its partial out-projection; the host sums the 4 partial
out-proj results per batch (the "all-reduce") and stacks batches.

v2 design (bf16 overhaul):
  - All matmuls in bf16: same PE streaming rate as float32r, but FWL halves
    the per-matmul LDWEIGHTS cost, DMA bytes halve, and DVE elementwise ops
    run at 2x on 16-bit.
  - Q^T/K^T/V stay SBUF-resident between projection and attention (no DRAM
    round trip).
  - RoPE rotate-half is a partition-shifted SBUF->SBUF DMA copy (the sign is
    folded into the host-built sin table), not a PE matmul.
  - V is transposed [d,t]->[t,d] with the DMA XBAR transpose, not PE.
  - Softmax denominators for all 4 heads of a query super-block accumulate
    into one [4, 512] PSUM tile via per-head one-hot ones columns, so one
    [4,512] reciprocal replaces 16 broadcast [128,512] reciprocals.
  - Phases B (attention) and C (out-proj) are merged, super-block-outer:
    each 512-query block's out-projection runs as soon as its softmax is
    normalized, overlapping y DMA writes with later attention.
  - QKV bias is applied on the Scalar engine during PSUM evacuation;
    the output bias is added on the host after the partial sum.
"""

import math
import os
import sys
import types

import numpy as np
import ml_dtypes

BF16NP = ml_dtypes.bfloat16

# ---------------------------------------------------------------------------
# NTFF profile hook (missing antenv.axon_hooks in this image). Reconstructed
# so run_bass_kernel_spmd(trace=True) can measure HW exec time.
# ---------------------------------------------------------------------------
try:
    import antenv

    if "antenv.axon_hooks" not in sys.modules:
        try:
            from trn_agent_boot.trn_boot import _ntff_profile_via_ctypes

            _hook = _ntff_profile_via_ctypes("/opt/axon/libaxon_pjrt.so")
        except Exception:
            _hook = None
        _m = types.ModuleType("antenv.axon_hooks")
        _m.get_axon_ntff_profile_hook = lambda: _hook
        _m.set_axon_ntff_profile_hook = lambda h: None
        sys.modules["antenv.axon_hooks"] = _m
        antenv.axon_hooks = _m
except Exception:
    pass

import concourse.bass as bass
import concourse.tile as tile
from concourse import bacc, mybir
from concourse.bass_utils import run_bass_kernel_spmd

# Problem constants (hardcoded per the task contract).
B = 2
T = 2048
DIM = 2048
H = 16
HD = 128                  # head_dim
G = 4                     # head groups (cores per batch)
HPG = H // G              # heads per group = 4
N_CORES = 8
SCALE = 1.0 / math.sqrt(HD)

F32 = mybir.dt.float32
BF16 = mybir.dt.bfloat16

TSL = 512                 # t-slice width in the projection phase
NTSL = T // TSL           # 4
QSB = 512                 # query super-block width in the attention phase
NSB = T // QSB            # 4
KC = 128                  # key chunk (partition dim)

LAST_EXEC_NS = None
LAST_RESULTS = None

_PROGRAM_CACHE = {}


def _build_program():
    nc = bacc.Bacc("TRN2", target_bir_lowering=False, debug=False,
                   num_devices=N_CORES)

    xT = nc.dram_tensor("xT", [DIM, T], BF16, kind="ExternalInput").ap()
    w_qkv = nc.dram_tensor("w_qkv_loc", [DIM, 3 * HPG * HD], BF16,
                           kind="ExternalInput").ap()
    b_cols = nc.dram_tensor("b_cols", [HD, 3 * HPG], F32,
                            kind="ExternalInput").ap()
    w_out = nc.dram_tensor("w_out_loc", [HPG * HD, DIM], BF16,
                           kind="ExternalInput").ap()
    cosT = nc.dram_tensor("cosT", [HD, T], BF16, kind="ExternalInput").ap()
    sinT = nc.dram_tensor("sinTs", [HD, T], BF16, kind="ExternalInput").ap()
    masks = nc.dram_tensor("masks_t", [KC, QSB // KC, QSB], BF16,
                           kind="ExternalInput").ap()
    y = nc.dram_tensor("y_part", [T, DIM], BF16, kind="ExternalOutput").ap()

    with tile.TileContext(nc) as tc:
        _emit(tc, nc, xT, w_qkv, b_cols, w_out, cosT, sinT, masks, y)

    nc.compile()
    return nc


def _emit(tc, nc, xT, w_qkv, b_cols_d, w_out, cosT_d, sinT_d, masks_d, y):
    from contextlib import ExitStack

    ctx = ExitStack()
    with ctx:
        ctx.enter_context(nc.allow_low_precision(
            reason="bf16 matmul operands and elementwise pipeline"))
        # ---------------- constants (live for the whole kernel) -----------
        consts = ctx.enter_context(tc.tile_pool(name="consts", bufs=1))
        bcols = consts.tile([HD, 3 * HPG], F32, tag="bcols")
        nc.gpsimd.dma_start(out=bcols, in_=b_cols_d)
        # ones4[:, h, :] is the [128, 4] one-hot stationary for head h: only
        # column h is ones, so head h's softmax-sum matmul lands in row h of
        # the shared [HPG, QSB] PSUM accumulator (other rows accumulate +0).
        ones4 = consts.tile([KC, HPG, HPG], BF16, tag="ones4")
        nc.vector.memset(ones4, 0.0)
        for h in range(HPG):
            nc.vector.memset(ones4[:, h, h:h + 1], 1.0)
        masks_sb = consts.tile([KC, QSB // KC, QSB], BF16, tag="masks")
        nc.gpsimd.dma_start(out=masks_sb, in_=masks_d)

        # QKV, attention output: SBUF-resident for the whole kernel.
        qkv_pool = ctx.enter_context(tc.tile_pool(name="qkv", bufs=1))
        qtr = [qkv_pool.tile([HD, T], BF16, tag=f"qtr{h}", name=f"qtr{h}")
               for h in range(HPG)]
        ktr = [qkv_pool.tile([HD, T], BF16, tag=f"ktr{h}", name=f"ktr{h}")
               for h in range(HPG)]
        vh = [qkv_pool.tile([KC, T // KC, HD], BF16, tag=f"vh{h}",
                            name=f"vh{h}")
              for h in range(HPG)]

        rope = ctx.enter_context(tc.tile_pool(name="rope", bufs=1))
        cosT = rope.tile([HD, T], BF16, tag="cosT")
        sinT = rope.tile([HD, T], BF16, tag="sinT")

        xT_r = xT.rearrange("(c p) t -> p c t", p=KC)        # [128, 16, T]
        w_r = w_qkv.rearrange("(c p) f -> p c f", p=KC)      # [128, 16, 1536]
        NKCH = DIM // KC                                     # 16 k-chunks

        w_out_r = w_out.rearrange("(c p) o -> p c o", p=KC)

        # ======================= Phase A: QKV + RoPE ======================
        with (
            tc.tile_pool(name="a_w", bufs=1) as a_w,
            tc.tile_pool(name="a_x", bufs=4) as a_x,
            tc.tile_pool(name="a_vb", bufs=3) as a_vb,
            tc.tile_pool(name="a_qb", bufs=5) as a_qb,
            tc.tile_pool(name="a_rot", bufs=5) as a_rot,
            tc.tile_pool(name="a_m1", bufs=5) as a_m1,
            tc.tile_pool(name="a_ps", bufs=8, space="PSUM") as a_ps,
        ):
            # Two HWDGE queues: SP (nc.sync) carries the latency-sensitive
            # small DMAs (rope rotations, V transposes) that are emitted
            # throughout phase A; the WAR-free bulk preloads go on the
            # Activation queue (nc.scalar) so they never head-of-line-block
            # them. xsl3 reuses xsl0's buffer (a WAR wait), so it is emitted
            # late on SP, where the wait is met by the time it reaches the
            # queue head.
            def load_xsl(tsl, q):
                t0 = tsl * TSL
                xs = a_x.tile([KC, NKCH, TSL], BF16, tag="xsl",
                              name=f"xsl{tsl}")
                for jj in range(4):
                    q.dma_start(
                        out=xs[:, jj * 4:(jj + 1) * 4, :],
                        in_=xT_r[:, jj * 4:(jj + 1) * 4, t0:t0 + TSL])
                return xs

            xsls = [load_xsl(0, nc.sync)]
            # w_qkv_loc is host-packed head-pair-major:
            # [hp0: q(2 heads), k, v | hp1: q, k, v], 256 cols per block.
            # Bulk loads ride the GpSimd queue: descriptor generation costs
            # ~0.7us of sequencer time per DMA, and GpSimd is otherwise idle
            # until phase B.
            w_all = a_w.tile([KC, NKCH, 3 * HPG * HD], BF16, tag="w_all")
            for kc in range(NKCH):
                nc.gpsimd.dma_start(out=w_all[:, kc, 0:768],
                                    in_=w_r[:, kc, 0:768])
            xsls.append(load_xsl(1, nc.gpsimd))
            for kc in range(NKCH):
                nc.gpsimd.dma_start(out=w_all[:, kc, 768:1536],
                                    in_=w_r[:, kc, 768:1536])
            nc.gpsimd.dma_start(out=cosT, in_=cosT_d)
            nc.gpsimd.dma_start(out=sinT, in_=sinT_d)
            # x is fully resident (4 buffers): no buffer-reusing DMA ever
            # transfers mid-phase. (A WAR-delayed load bursts through the
            # shared DMA rings mid-phase and everything whose completion
            # counter sits behind it on the ring stalls with it.)
            xsls.append(load_xsl(2, nc.gpsimd))
            xsls.append(load_xsl(3, nc.gpsimd))

            for tsl in range(NTSL):
                t0 = tsl * TSL
                xsl = xsls[tsl]

                # Last t-slice: process heads (2,3) first so the final psum
                # evac chain belongs to heads (0,1); phase B starts with head
                # 2, whose data (and psum banks) free up first.
                hp_order = (1, 0) if tsl == NTSL - 1 else (0, 1)
                for hp in hp_order:
                    heads = (2 * hp, 2 * hp + 1)
                    outs = [(h, kind) for h in heads for kind in range(3)]
                    # kc-outer: six psum accumulators advance together, so the
                    # PE tracks weight-chunk DMA arrival instead of stalling on
                    # the full weight load.
                    pstiles = {}
                    for (h, kind) in outs:
                        pstiles[(h, kind)] = a_ps.tile(
                            [HD, TSL], F32, tag="ps_qkv",
                            name=f"ps_{tsl}_{h}_{kind}")
                    for kc in range(NKCH):
                        for (h, kind) in outs:
                            feat0 = (h // 2) * 768 + kind * 256 + (h % 2) * HD
                            nc.tensor.matmul(
                                pstiles[(h, kind)],
                                w_all[:, kc, feat0:feat0 + HD],
                                xsl[:, kc, :],
                                start=(kc == 0), stop=(kc == NKCH - 1),
                            )
                    # Staged evacuation: each stage fans out across its
                    # engine queue before the next depends on it, so no
                    # queue head-of-line-blocks another (the serial
                    # per-chain version stalled phase-B's PSUM reuse).
                    # 1) V evacs (DVE) -> 2) q/k evacs (Scalar) ->
                    # 3) V transposes -> 4) rotation DMAs ->
                    # 5) m1 muls (overlap the rotations in flight) ->
                    # 6) m2 muls -> 7) adds into qtr/ktr.
                    vbs, qbs, qrots, m1s = {}, {}, {}, {}
                    for h in heads:
                        vb = a_vb.tile([HD, TSL], BF16, tag="vb")
                        nc.vector.tensor_scalar_add(
                            vb, pstiles[(h, 2)],
                            bcols[:, 2 * HPG + h:2 * HPG + h + 1])
                        vbs[h] = vb
                    for h in heads:
                        for kind in (0, 1):
                            qb = a_qb.tile([HD, TSL], F32, tag="qb")
                            nc.scalar.activation(
                                qb, pstiles[(h, kind)],
                                mybir.ActivationFunctionType.Identity,
                                bias=bcols[:, kind * HPG + h:
                                           kind * HPG + h + 1])
                            qbs[(h, kind)] = qb
                    for h in heads:
                        nc.sync.dma_start_transpose(
                            out=vh[h][:, tsl * (TSL // KC):
                                      (tsl + 1) * (TSL // KC), :],
                            in_=vbs[h])
                    half = HD // 2
                    for h in heads:
                        for kind in (0, 1):
                            qb = qbs[(h, kind)]
                            # rotate-half: partition-shifted SBUF->SBUF copy
                            # (sign folded into the host-built sin table)
                            qrot = a_rot.tile([HD, TSL], F32, tag="qrot")
                            nc.sync.dma_start(out=qrot[0:half, :],
                                              in_=qb[half:HD, :])
                            nc.sync.dma_start(out=qrot[half:HD, :],
                                              in_=qb[0:half, :])
                            qrots[(h, kind)] = qrot
                    for h in heads:
                        for kind in (0, 1):
                            m1 = a_m1.tile([HD, TSL], F32, tag="m1")
                            nc.vector.tensor_mul(m1, qbs[(h, kind)],
                                                 cosT[:, t0:t0 + TSL])
                            m1s[(h, kind)] = m1
                    for h in heads:
                        for kind in (0, 1):
                            # m2 in place on qrot (saves an SBUF ring)
                            qrot = qrots[(h, kind)]
                            nc.vector.tensor_mul(qrot, qrot,
                                                 sinT[:, t0:t0 + TSL])
                            dst = qtr[h] if kind == 0 else ktr[h]
                            nc.vector.tensor_add(dst[:, t0:t0 + TSL],
                                                 m1s[(h, kind)], qrot)



        # ================= Phase B+C: attention + out-proj ================
        # Software-pipelined emission (the Tensor queue executes in order, so
        # emission order IS the PE schedule):
        #   - head order [2,3,0,1] matches the phase-A evac order above;
        #   - softmax-sum matmuls are emitted one gpair late, so the next
        #     super-block's first sum matmul never head-of-line-blocks on the
        #     previous block's reciprocal (same PSUM bank);
        #   - each super-block's out-proj units are interleaved into the NEXT
        #     super-block's attention gpairs (starting at gpair 4, giving the
        #     normalize chain time to finish); the last block's units drain at
        #     the end.
        HEAD_ORDER = (2, 3, 0, 1)
        NOB = DIM // 512
        NU = (QSB // KC) * NOB                    # 16 out-proj units per sb
        # out-proj weights land in the space freed by phase A; the first
        # consumer (an out-proj unit of sb0) runs ~25us into phase B.
        c_w = ctx.enter_context(tc.tile_pool(name="c_w", bufs=1))
        wo = c_w.tile([KC, HPG, DIM], BF16, tag="wo")
        for hc in range(HPG):
            nc.gpsimd.dma_start(out=wo[:, hc, :], in_=w_out_r[:, hc, :])
        with (
            tc.tile_pool(name="b_pt", bufs=4) as b_pt,
            tc.tile_pool(name="b_ot", bufs=2) as b_ot,
            tc.tile_pool(name="b_sm", bufs=2) as b_sm,
            tc.tile_pool(name="c_sb", bufs=4) as c_sb,
            tc.tile_pool(name="b_ps_s", bufs=2, space="PSUM") as b_ps_s,
            tc.tile_pool(name="b_ps_o", bufs=1, space="PSUM") as b_ps_o,
            tc.tile_pool(name="b_ps_sum", bufs=1, space="PSUM") as b_ps_sum,
            tc.tile_pool(name="c_ps", bufs=2, space="PSUM") as c_ps,
        ):
            def emit_c_unit(csb, u, otn_map):
                tb, ob = divmod(u, NOB)
                tt0 = tb * KC
                o0 = ob * 512
                ps_y = c_ps.tile([KC, 512], F32, tag="ps_y")
                # accumulate in HEAD_ORDER: the first otn to be normalized is
                # needed first, shrinking the drain stall after the last sb
                for i, hc in enumerate(HEAD_ORDER):
                    nc.tensor.matmul(
                        ps_y, otn_map[hc][:, tt0:tt0 + KC],
                        wo[:, hc, o0:o0 + 512],
                        start=(i == 0), stop=(i == HPG - 1),
                    )
                ys = c_sb.tile([KC, 512], BF16, tag="ys")
                if u % 2 == 0:
                    nc.scalar.activation(
                        ys, ps_y, mybir.ActivationFunctionType.Identity)
                else:
                    nc.vector.tensor_copy(ys, ps_y)
                r0 = csb * QSB + tt0
                nc.sync.dma_start(out=y[r0:r0 + KC, o0:o0 + 512], in_=ys)

            otn_prev = None
            prev_sb = None
            norm_queue = []
            for sb in range(NSB):
                q0 = sb * QSB
                nk = (sb + 1) * (QSB // KC)       # causal key chunks
                ngp = nk // 2
                total_gp = HPG * ngp
                # schedule prev sb's NU out-proj units over gpair slots >= 8
                # (by slot 8 the previous block's normalize chain is done)
                slot_units = {}
                if otn_prev is not None:
                    lo = 8 if total_gp > 9 else total_gp - 1
                    span = max(total_gp - lo, 1)
                    for u in range(NU):
                        s = lo + (u * span) // NU
                        slot_units.setdefault(min(s, total_gp - 1),
                                              []).append(u)

                ps_sum = b_ps_sum.tile([HPG, QSB], F32, tag="ps_sum")
                pending_norms = list(norm_queue)
                norm_queue = []
                sum_queue = []      # sums run two gpairs late: the previous
                sum_first = True    # reciprocal is long done by then

                def flush_sum(last):
                    nonlocal sum_first
                    while sum_queue and (len(sum_queue) > 2 or last):
                        fh, fpt, fc0s = sum_queue.pop(0)
                        for j in range(2):
                            nc.tensor.matmul(
                                ps_sum[:, fc0s[j]:], ones4[:, fh, :],
                                fpt[:, j, fc0s[j]:],
                                start=(sum_first and j == 0),
                                stop=(last and not sum_queue and j == 1),
                            )
                        sum_first = False

                otu = {}
                gslot = 0
                for h in HEAD_ORDER:
                    ps_o = b_ps_o.tile([HD, QSB], F32, tag="ps_o")
                    for gpair in range(ngp):
                        k0 = 2 * gpair
                        # Diagonal chunks (dj >= 0) only attend to queries
                        # q >= dj*128: trim the streamed column range of the
                        # S/O/sum matmuls, exp, and mask to the valid part.
                        # (The trimmed-off region of pt/psum is stale but is
                        # never read.)
                        djs = [(k0 + j) - (nk - QSB // KC) for j in range(2)]
                        c0s = [max(dj, 0) * KC for dj in djs]
                        cmin = min(c0s)
                        ps_st = b_ps_s.tile([KC, 2, QSB], F32, tag="ps_st")
                        for j in range(2):
                            c0 = c0s[j]
                            nc.tensor.matmul(
                                ps_st[:, j, c0:],
                                ktr[h][:, (k0 + j) * KC:(k0 + j + 1) * KC],
                                qtr[h][:, q0 + c0:q0 + QSB],
                                start=True, stop=True,
                            )
                        for u in slot_units.get(gslot, ()):
                            emit_c_unit(prev_sb, u, otn_prev)
                        pt = b_pt.tile([KC, 2, QSB], BF16, tag="pt")
                        nc.scalar.activation(
                            pt[:, :, cmin:], ps_st[:, :, cmin:],
                            mybir.ActivationFunctionType.Exp, scale=SCALE)
                        for j in range(2):
                            dj = djs[j]
                            if dj >= 0:
                                c0 = c0s[j]
                                nc.vector.tensor_mul(
                                    pt[:, j, c0:], pt[:, j, c0:],
                                    masks_sb[:, dj, c0:])
                        for j in range(2):
                            kci = k0 + j
                            nc.tensor.matmul(
                                ps_o[:, c0s[j]:], vh[h][:, kci, :],
                                pt[:, j, c0s[j]:],
                                start=(kci == 0), stop=(kci == nk - 1),
                            )
                        sum_queue.append((h, pt, c0s))
                        flush_sum(False)
                        gslot += 1
                        if pending_norms and gslot >= 1:
                            pending_norms.pop(0)()
                    # evacuate unnormalized O^T (bf16); normalized after the
                    # batched reciprocal below.
                    ou = b_ot.tile([HD, QSB], BF16, tag=f"otu{h}")
                    nc.vector.tensor_copy(ou, ps_o)
                    otu[h] = ou
                flush_sum(True)

                # batched reciprocal: one [4, 512] op for all heads
                rsums = b_sm.tile([HPG, QSB], F32, tag="rsums")
                nc.vector.reciprocal(rsums, ps_sum)
                # partition_broadcast requires its input at partition 0:
                # stage all 4 rows there first (the DMAs run concurrently).
                r1s = {}
                for h in HEAD_ORDER:
                    r1 = b_sm.tile([1, QSB], F32, tag=f"r1{h}")
                    nc.sync.dma_start(out=r1, in_=rsums[h:h + 1, :])
                    r1s[h] = r1
                # The per-head normalize (broadcast + multiply) is DEFERRED:
                # emitted between the next super-block's early gpairs so the
                # DVE queue interleaves it with new attention work instead of
                # head-of-line-blocking on the whole chain.
                otn = {}

                def make_norm(h, r1, ou, dst):
                    def emit():
                        rb = b_sm.tile([KC, QSB], F32, tag="rb")
                        nc.gpsimd.partition_broadcast(rb, r1, channels=KC)
                        on = b_ot.tile([HD, QSB], BF16, tag=f"otn{h}")
                        nc.vector.tensor_mul(on, ou, rb)
                        dst[h] = on
                    return emit

                norm_queue = [make_norm(h, r1s[h], otu[h], otn)
                              for h in HEAD_ORDER]
                if sb == NSB - 1:
                    for fn in norm_queue:
                        fn()
                    norm_queue = []

                otn_prev = otn
                prev_sb = sb

            # drain the last super-block's out-proj units
            for u in range(NU):
                emit_c_unit(prev_sb, u, otn_prev)


# ---------------------------------------------------------------------------
# Host-side input prep
# ---------------------------------------------------------------------------


def _rope_tables():
    inv_freq = 1.0 / (10000.0 ** (np.arange(0, HD, 2, dtype=np.float64) / HD))
    t = np.arange(T, dtype=np.float64)
    freqs = np.outer(t, inv_freq)                     # [T, 64]
    emb = np.concatenate([freqs, freqs], axis=-1)     # [T, 128]
    cosT = np.cos(emb).T.astype(np.float32)           # [128, T]
    sinT = np.sin(emb).T.astype(np.float32)
    # rotate_half(x) = [-x2, x1]; the device computes qrot = [x2, x1], so
    # fold the sign of the first half into the sin table.
    sinT[:HD // 2, :] *= -1.0
    return (np.ascontiguousarray(cosT.astype(BF16NP)),
            np.ascontiguousarray(sinT.astype(BF16NP)))


def _masks_t():
    # masks[r, j, c] = 1 if c >= j*128 + r  (causal mask for the diagonal
    # 512-wide block, per 128-key chunk j)
    r = np.arange(KC)[:, None, None]
    j = np.arange(QSB // KC)[None, :, None]
    c = np.arange(QSB)[None, None, :]
    return (c >= j * KC + r).astype(BF16NP)


def kernel(x, w_qkv, b_qkv, w_out, b_out):
    global LAST_EXEC_NS, LAST_RESULTS

    x = np.asarray(x, dtype=np.float32)
    w_qkv = np.asarray(w_qkv, dtype=np.float32)
    b_qkv = np.asarray(b_qkv, dtype=np.float32)
    w_out = np.asarray(w_out, dtype=np.float32)
    b_out = np.asarray(b_out, dtype=np.float32)

    if "prog" not in _PROGRAM_CACHE:
        _PROGRAM_CACHE["prog"] = _build_program()
    nc = _PROGRAM_CACHE["prog"]

    cosT, sinT = _rope_tables()
    masks = _masks_t()

    xTs = [np.ascontiguousarray(x[b].T.astype(BF16NP)) for b in range(B)]
    in_maps = []
    for c in range(N_CORES):
        b = c // G
        g = c % G
        f0 = g * HPG * HD
        f1 = (g + 1) * HPG * HD
        w_loc = np.ascontiguousarray(np.concatenate(
            [w_qkv[:, base + f0 + hp * 256: base + f0 + (hp + 1) * 256]
             for hp in range(HPG // 2)
             for base in (0, DIM, 2 * DIM)], axis=1).astype(BF16NP))
        b_loc = np.concatenate(
            [b_qkv[f0:f1], b_qkv[DIM + f0:DIM + f1],
             b_qkv[2 * DIM + f0:2 * DIM + f1]])
        b_cols = np.ascontiguousarray(
            b_loc.reshape(3 * HPG, HD).T).astype(np.float32)
        w_out_loc = np.ascontiguousarray(w_out[f0:f1, :].astype(BF16NP))
        in_maps.append({
            "xT": xTs[b],
            "w_qkv_loc": w_loc,
            "b_cols": b_cols,
            "w_out_loc": w_out_loc,
            "cosT": cosT,
            "sinTs": sinT,
            "masks_t": masks,
        })

    trace = bool(os.environ.get("BASS_KERNEL_TRACE"))
    res = run_bass_kernel_spmd(nc, in_maps, list(range(N_CORES)), trace=trace)
    LAST_EXEC_NS = res.exec_time_ns
    LAST_RESULTS = res

    out = np.empty((B, T, DIM), dtype=np.float32)
    for b in range(B):
        acc = res.results[4 * b]["y_part"].astype(np.float32)
        for g in range(1, G):
            acc = acc + res.results[4 * b + g]["y_part"].astype(np.float32)
        out[b] = acc + b_out[None, :]
    return out


# revision 31
# speedup vs baseline: 1.0400x; 1.0400x over previous
"""Causal self-attention (B=2, T=2048, dim=2048, 16 heads, RoPE) on 8 trn2
NeuronCores.

Sharding: core c handles batch b = c//4 and head group g = c%4 (4 heads each,
tensor-parallel over heads). Each core computes QKV projection + RoPE +
causal attention + its partial out-projection; the host sums the 4 partial
out-proj results per batch (the "all-reduce") and stacks batches.

v2 design (bf16 overhaul):
  - All matmuls in bf16: same PE streaming rate as float32r, but FWL halves
    the per-matmul LDWEIGHTS cost, DMA bytes halve, and DVE elementwise ops
    run at 2x on 16-bit.
  - Q^T/K^T/V stay SBUF-resident between projection and attention (no DRAM
    round trip).
  - RoPE rotate-half is a partition-shifted SBUF->SBUF DMA copy (the sign is
    folded into the host-built sin table), not a PE matmul.
  - V is transposed [d,t]->[t,d] with the DMA XBAR transpose, not PE.
  - Softmax denominators for all 4 heads of a query super-block accumulate
    into one [4, 512] PSUM tile via per-head one-hot ones columns, so one
    [4,512] reciprocal replaces 16 broadcast [128,512] reciprocals.
  - Phases B (attention) and C (out-proj) are merged, super-block-outer:
    each 512-query block's out-projection runs as soon as its softmax is
    normalized, overlapping y DMA writes with later attention.
  - QKV bias is applied on the Scalar engine during PSUM evacuation;
    the output bias is added on the host after the partial sum.
"""

import math
import os
import sys
import types

import numpy as np
import ml_dtypes

BF16NP = ml_dtypes.bfloat16

# ---------------------------------------------------------------------------
# NTFF profile hook (missing antenv.axon_hooks in this image). Reconstructed
# so run_bass_kernel_spmd(trace=True) can measure HW exec time.
# ---------------------------------------------------------------------------
try:
    import antenv

    if "antenv.axon_hooks" not in sys.modules:
        try:
            from trn_agent_boot.trn_boot import _ntff_profile_via_ctypes

            _hook = _ntff_profile_via_ctypes("/opt/axon/libaxon_pjrt.so")
        except Exception:
            _hook = None
        _m = types.ModuleType("antenv.axon_hooks")
        _m.get_axon_ntff_profile_hook = lambda: _hook
        _m.set_axon_ntff_profile_hook = lambda h: None
        sys.modules["antenv.axon_hooks"] = _m
        antenv.axon_hooks = _m
except Exception:
    pass

import concourse.bass as bass
import concourse.tile as tile
from concourse import bacc, mybir
from concourse.bass_utils import run_bass_kernel_spmd

# Problem constants (hardcoded per the task contract).
B = 2
T = 2048
DIM = 2048
H = 16
HD = 128                  # head_dim
G = 4                     # head groups (cores per batch)
HPG = H // G              # heads per group = 4
N_CORES = 8
SCALE = 1.0 / math.sqrt(HD)

F32 = mybir.dt.float32
BF16 = mybir.dt.bfloat16

TSL = 512                 # t-slice width in the projection phase
NTSL = T // TSL           # 4
QSB = 512                 # query super-block width in the attention phase
NSB = T // QSB            # 4
KC = 128                  # key chunk (partition dim)

LAST_EXEC_NS = None
LAST_RESULTS = None

_PROGRAM_CACHE = {}


def _build_program():
    nc = bacc.Bacc("TRN2", target_bir_lowering=False, debug=False,
                   num_devices=N_CORES)

    xT = nc.dram_tensor("xT", [DIM, T], BF16, kind="ExternalInput").ap()
    w_qkv = nc.dram_tensor("w_qkv_loc", [DIM, 3 * HPG * HD], BF16,
                           kind="ExternalInput").ap()
    b_cols = nc.dram_tensor("b_cols", [HD, 3 * HPG], F32,
                            kind="ExternalInput").ap()
    w_out = nc.dram_tensor("w_out_loc", [HPG * HD, DIM], BF16,
                           kind="ExternalInput").ap()
    cosT = nc.dram_tensor("cosT", [HD, T], BF16, kind="ExternalInput").ap()
    sinT = nc.dram_tensor("sinTs", [HD, T], BF16, kind="ExternalInput").ap()
    masks = nc.dram_tensor("masks_t", [KC, QSB // KC, QSB], BF16,
                           kind="ExternalInput").ap()
    y = nc.dram_tensor("y_part", [T, DIM], BF16, kind="ExternalOutput").ap()

    with tile.TileContext(nc) as tc:
        _emit(tc, nc, xT, w_qkv, b_cols, w_out, cosT, sinT, masks, y)

    nc.compile()
    return nc


def _emit(tc, nc, xT, w_qkv, b_cols_d, w_out, cosT_d, sinT_d, masks_d, y):
    from contextlib import ExitStack

    ctx = ExitStack()
    with ctx:
        ctx.enter_context(nc.allow_low_precision(
            reason="bf16 matmul operands and elementwise pipeline"))
        # ---------------- constants (live for the whole kernel) -----------
        consts = ctx.enter_context(tc.tile_pool(name="consts", bufs=1))
        bcols = consts.tile([HD, 3 * HPG], F32, tag="bcols")
        nc.gpsimd.dma_start(out=bcols, in_=b_cols_d)
        # ones4[:, h, :] is the [128, 4] one-hot stationary for head h: only
        # column h is ones, so head h's softmax-sum matmul lands in row h of
        # the shared [HPG, QSB] PSUM accumulator (other rows accumulate +0).
        ones4 = consts.tile([KC, HPG, HPG], BF16, tag="ones4")
        nc.vector.memset(ones4, 0.0)
        for h in range(HPG):
            nc.vector.memset(ones4[:, h, h:h + 1], 1.0)
        masks_sb = consts.tile([KC, QSB // KC, QSB], BF16, tag="masks")
        nc.gpsimd.dma_start(out=masks_sb, in_=masks_d)

        # QKV, attention output: SBUF-resident for the whole kernel.
        qkv_pool = ctx.enter_context(tc.tile_pool(name="qkv", bufs=1))
        qtr = [qkv_pool.tile([HD, T], BF16, tag=f"qtr{h}", name=f"qtr{h}")
               for h in range(HPG)]
        ktr = [qkv_pool.tile([HD, T], BF16, tag=f"ktr{h}", name=f"ktr{h}")
               for h in range(HPG)]
        vh = [qkv_pool.tile([KC, T // KC, HD], BF16, tag=f"vh{h}",
                            name=f"vh{h}")
              for h in range(HPG)]

        rope = ctx.enter_context(tc.tile_pool(name="rope", bufs=1))
        cosT = rope.tile([HD, T], BF16, tag="cosT")
        sinT = rope.tile([HD, T], BF16, tag="sinT")

        xT_r = xT.rearrange("(c p) t -> p c t", p=KC)        # [128, 16, T]
        w_r = w_qkv.rearrange("(c p) f -> p c f", p=KC)      # [128, 16, 1536]
        NKCH = DIM // KC                                     # 16 k-chunks

        w_out_r = w_out.rearrange("(c p) o -> p c o", p=KC)

        # ======================= Phase A: QKV + RoPE ======================
        with (
            tc.tile_pool(name="a_w", bufs=1) as a_w,
            tc.tile_pool(name="a_x", bufs=4) as a_x,
            tc.tile_pool(name="a_vb", bufs=3) as a_vb,
            tc.tile_pool(name="a_qb", bufs=5) as a_qb,
            tc.tile_pool(name="a_rot", bufs=5) as a_rot,
            tc.tile_pool(name="a_m1", bufs=5) as a_m1,
            tc.tile_pool(name="a_ps", bufs=8, space="PSUM") as a_ps,
        ):
            # Two HWDGE queues: SP (nc.sync) carries the latency-sensitive
            # small DMAs (rope rotations, V transposes) that are emitted
            # throughout phase A; the WAR-free bulk preloads go on the
            # Activation queue (nc.scalar) so they never head-of-line-block
            # them. xsl3 reuses xsl0's buffer (a WAR wait), so it is emitted
            # late on SP, where the wait is met by the time it reaches the
            # queue head.
            def load_xsl(tsl, q):
                t0 = tsl * TSL
                xs = a_x.tile([KC, NKCH, TSL], BF16, tag="xsl",
                              name=f"xsl{tsl}")
                for jj in range(4):
                    q.dma_start(
                        out=xs[:, jj * 4:(jj + 1) * 4, :],
                        in_=xT_r[:, jj * 4:(jj + 1) * 4, t0:t0 + TSL])
                return xs

            xsls = [load_xsl(0, nc.sync)]
            # Bulk preloads are split across BOTH spare queues so no single
            # per-queue DMA-completion counter spans the whole ~16MB (a
            # coarse semaphore wait on "bulk queue drained" stalled the
            # evac pipeline until the last byte landed): x slices ride the
            # Activation queue (descriptor gen finishes before the first
            # evac needs it), weights/rope/masks ride GpSimd, rope tables
            # first. x is fully resident (4 buffers): no WAR loads.
            nc.gpsimd.dma_start(out=cosT, in_=cosT_d)
            nc.gpsimd.dma_start(out=sinT, in_=sinT_d)
            xsls.append(load_xsl(1, nc.scalar))
            # w_qkv_loc is host-packed head-pair-major:
            # [hp0: q(2 heads), k, v | hp1: q, k, v], 256 cols per block.
            w_all = a_w.tile([KC, NKCH, 3 * HPG * HD], BF16, tag="w_all")
            for kc in range(NKCH):
                nc.gpsimd.dma_start(out=w_all[:, kc, 0:768],
                                    in_=w_r[:, kc, 0:768])
            xsls.append(load_xsl(2, nc.scalar))
            for kc in range(NKCH):
                nc.gpsimd.dma_start(out=w_all[:, kc, 768:1536],
                                    in_=w_r[:, kc, 768:1536])
            xsls.append(load_xsl(3, nc.scalar))

            for tsl in range(NTSL):
                t0 = tsl * TSL
                xsl = xsls[tsl]

                # Last t-slice: process heads (2,3) first so the final psum
                # evac chain belongs to heads (0,1); phase B starts with head
                # 2, whose data (and psum banks) free up first.
                hp_order = (1, 0) if tsl == NTSL - 1 else (0, 1)
                for hp in hp_order:
                    heads = (2 * hp, 2 * hp + 1)
                    outs = [(h, kind) for h in heads for kind in range(3)]
                    # kc-outer: six psum accumulators advance together, so the
                    # PE tracks weight-chunk DMA arrival instead of stalling on
                    # the full weight load.
                    pstiles = {}
                    for (h, kind) in outs:
                        pstiles[(h, kind)] = a_ps.tile(
                            [HD, TSL], F32, tag="ps_qkv",
                            name=f"ps_{tsl}_{h}_{kind}")
                    for kc in range(NKCH):
                        for (h, kind) in outs:
                            feat0 = (h // 2) * 768 + kind * 256 + (h % 2) * HD
                            nc.tensor.matmul(
                                pstiles[(h, kind)],
                                w_all[:, kc, feat0:feat0 + HD],
                                xsl[:, kc, :],
                                start=(kc == 0), stop=(kc == NKCH - 1),
                            )
                    # Staged evacuation: each stage fans out across its
                    # engine queue before the next depends on it, so no
                    # queue head-of-line-blocks another (the serial
                    # per-chain version stalled phase-B's PSUM reuse).
                    # 1) V evacs (DVE) -> 2) q/k evacs (Scalar) ->
                    # 3) V transposes -> 4) rotation DMAs ->
                    # 5) m1 muls (overlap the rotations in flight) ->
                    # 6) m2 muls -> 7) adds into qtr/ktr.
                    vbs, qbs, qrots, m1s = {}, {}, {}, {}
                    for h in heads:
                        vb = a_vb.tile([HD, TSL], BF16, tag="vb")
                        nc.vector.tensor_scalar_add(
                            vb, pstiles[(h, 2)],
                            bcols[:, 2 * HPG + h:2 * HPG + h + 1])
                        vbs[h] = vb
                    for h in heads:
                        for kind in (0, 1):
                            qb = a_qb.tile([HD, TSL], F32, tag="qb")
                            nc.scalar.activation(
                                qb, pstiles[(h, kind)],
                                mybir.ActivationFunctionType.Identity,
                                bias=bcols[:, kind * HPG + h:
                                           kind * HPG + h + 1])
                            qbs[(h, kind)] = qb
                    for h in heads:
                        nc.sync.dma_start_transpose(
                            out=vh[h][:, tsl * (TSL // KC):
                                      (tsl + 1) * (TSL // KC), :],
                            in_=vbs[h])
                    half = HD // 2
                    for h in heads:
                        for kind in (0, 1):
                            qb = qbs[(h, kind)]
                            # rotate-half: partition-shifted SBUF->SBUF copy
                            # (sign folded into the host-built sin table)
                            qrot = a_rot.tile([HD, TSL], F32, tag="qrot")
                            nc.sync.dma_start(out=qrot[0:half, :],
                                              in_=qb[half:HD, :])
                            nc.sync.dma_start(out=qrot[half:HD, :],
                                              in_=qb[0:half, :])
                            qrots[(h, kind)] = qrot
                    for h in heads:
                        for kind in (0, 1):
                            m1 = a_m1.tile([HD, TSL], F32, tag="m1")
                            nc.vector.tensor_mul(m1, qbs[(h, kind)],
                                                 cosT[:, t0:t0 + TSL])
                            m1s[(h, kind)] = m1
                    for h in heads:
                        for kind in (0, 1):
                            # m2 in place on qrot (saves an SBUF ring)
                            qrot = qrots[(h, kind)]
                            nc.vector.tensor_mul(qrot, qrot,
                                                 sinT[:, t0:t0 + TSL])
                            dst = qtr[h] if kind == 0 else ktr[h]
                            nc.vector.tensor_add(dst[:, t0:t0 + TSL],
                                                 m1s[(h, kind)], qrot)



        # ================= Phase B+C: attention + out-proj ================
        # Software-pipelined emission (the Tensor queue executes in order, so
        # emission order IS the PE schedule):
        #   - head order [2,3,0,1] matches the phase-A evac order above;
        #   - softmax-sum matmuls are emitted one gpair late, so the next
        #     super-block's first sum matmul never head-of-line-blocks on the
        #     previous block's reciprocal (same PSUM bank);
        #   - each super-block's out-proj units are interleaved into the NEXT
        #     super-block's attention gpairs (starting at gpair 4, giving the
        #     normalize chain time to finish); the last block's units drain at
        #     the end.
        HEAD_ORDER = (2, 3, 0, 1)
        NOB = DIM // 512
        NU = (QSB // KC) * NOB                    # 16 out-proj units per sb
        # out-proj weights land in the space freed by phase A; the first
        # consumer (an out-proj unit of sb0) runs ~25us into phase B.
        c_w = ctx.enter_context(tc.tile_pool(name="c_w", bufs=1))
        wo = c_w.tile([KC, HPG, DIM], BF16, tag="wo")
        for hc in range(HPG):
            nc.gpsimd.dma_start(out=wo[:, hc, :], in_=w_out_r[:, hc, :])
        with (
            tc.tile_pool(name="b_pt", bufs=4) as b_pt,
            tc.tile_pool(name="b_ot", bufs=2) as b_ot,
            tc.tile_pool(name="b_sm", bufs=2) as b_sm,
            tc.tile_pool(name="c_sb", bufs=4) as c_sb,
            tc.tile_pool(name="b_ps_s", bufs=2, space="PSUM") as b_ps_s,
            tc.tile_pool(name="b_ps_o", bufs=1, space="PSUM") as b_ps_o,
            tc.tile_pool(name="b_ps_sum", bufs=1, space="PSUM") as b_ps_sum,
            tc.tile_pool(name="c_ps", bufs=2, space="PSUM") as c_ps,
        ):
            def emit_c_unit(csb, u, otn_map):
                tb, ob = divmod(u, NOB)
                tt0 = tb * KC
                o0 = ob * 512
                ps_y = c_ps.tile([KC, 512], F32, tag="ps_y")
                # accumulate in HEAD_ORDER: the first otn to be normalized is
                # needed first, shrinking the drain stall after the last sb
                for i, hc in enumerate(HEAD_ORDER):
                    nc.tensor.matmul(
                        ps_y, otn_map[hc][:, tt0:tt0 + KC],
                        wo[:, hc, o0:o0 + 512],
                        start=(i == 0), stop=(i == HPG - 1),
                    )
                ys = c_sb.tile([KC, 512], BF16, tag="ys")
                if u % 2 == 0:
                    nc.scalar.activation(
                        ys, ps_y, mybir.ActivationFunctionType.Identity)
                else:
                    nc.vector.tensor_copy(ys, ps_y)
                r0 = csb * QSB + tt0
                nc.sync.dma_start(out=y[r0:r0 + KC, o0:o0 + 512], in_=ys)

            otn_prev = None
            prev_sb = None
            norm_queue = []
            for sb in range(NSB):
                q0 = sb * QSB
                nk = (sb + 1) * (QSB // KC)       # causal key chunks
                ngp = nk // 2
                total_gp = HPG * ngp
                # schedule prev sb's NU out-proj units over gpair slots >= 8
                # (by slot 8 the previous block's normalize chain is done)
                slot_units = {}
                if otn_prev is not None:
                    lo = 8 if total_gp > 9 else total_gp - 1
                    span = max(total_gp - lo, 1)
                    for u in range(NU):
                        s = lo + (u * span) // NU
                        slot_units.setdefault(min(s, total_gp - 1),
                                              []).append(u)

                ps_sum = b_ps_sum.tile([HPG, QSB], F32, tag="ps_sum")
                pending_norms = list(norm_queue)
                norm_queue = []
                sum_queue = []      # sums run two gpairs late: the previous
                sum_first = True    # reciprocal is long done by then

                def flush_sum(last):
                    nonlocal sum_first
                    while sum_queue and (len(sum_queue) > 2 or last):
                        fh, fpt, fc0s = sum_queue.pop(0)
                        for j in range(2):
                            nc.tensor.matmul(
                                ps_sum[:, fc0s[j]:], ones4[:, fh, :],
                                fpt[:, j, fc0s[j]:],
                                start=(sum_first and j == 0),
                                stop=(last and not sum_queue and j == 1),
                            )
                        sum_first = False

                otu = {}
                gslot = 0
                for h in HEAD_ORDER:
                    ps_o = b_ps_o.tile([HD, QSB], F32, tag="ps_o")
                    for gpair in range(ngp):
                        k0 = 2 * gpair
                        # Diagonal chunks (dj >= 0) only attend to queries
                        # q >= dj*128: trim the streamed column range of the
                        # S/O/sum matmuls, exp, and mask to the valid part.
                        # (The trimmed-off region of pt/psum is stale but is
                        # never read.)
                        djs = [(k0 + j) - (nk - QSB // KC) for j in range(2)]
                        c0s = [max(dj, 0) * KC for dj in djs]
                        cmin = min(c0s)
                        ps_st = b_ps_s.tile([KC, 2, QSB], F32, tag="ps_st")
                        for j in range(2):
                            c0 = c0s[j]
                            nc.tensor.matmul(
                                ps_st[:, j, c0:],
                                ktr[h][:, (k0 + j) * KC:(k0 + j + 1) * KC],
                                qtr[h][:, q0 + c0:q0 + QSB],
                                start=True, stop=True,
                            )
                        for u in slot_units.get(gslot, ()):
                            emit_c_unit(prev_sb, u, otn_prev)
                        pt = b_pt.tile([KC, 2, QSB], BF16, tag="pt")
                        nc.scalar.activation(
                            pt[:, :, cmin:], ps_st[:, :, cmin:],
                            mybir.ActivationFunctionType.Exp, scale=SCALE)
                        for j in range(2):
                            dj = djs[j]
                            if dj >= 0:
                                c0 = c0s[j]
                                nc.vector.tensor_mul(
                                    pt[:, j, c0:], pt[:, j, c0:],
                                    masks_sb[:, dj, c0:])
                        for j in range(2):
                            kci = k0 + j
                            nc.tensor.matmul(
                                ps_o[:, c0s[j]:], vh[h][:, kci, :],
                                pt[:, j, c0s[j]:],
                                start=(kci == 0), stop=(kci == nk - 1),
                            )
                        sum_queue.append((h, pt, c0s))
                        flush_sum(False)
                        gslot += 1
                        if pending_norms and gslot >= 1:
                            pending_norms.pop(0)()
                    # evacuate unnormalized O^T (bf16); normalized after the
                    # batched reciprocal below.
                    ou = b_ot.tile([HD, QSB], BF16, tag=f"otu{h}")
                    nc.vector.tensor_copy(ou, ps_o)
                    otu[h] = ou
                flush_sum(True)

                # batched reciprocal: one [4, 512] op for all heads
                rsums = b_sm.tile([HPG, QSB], F32, tag="rsums")
                nc.vector.reciprocal(rsums, ps_sum)
                # partition_broadcast requires its input at partition 0:
                # stage all 4 rows there first (the DMAs run concurrently).
                r1s = {}
                for h in HEAD_ORDER:
                    r1 = b_sm.tile([1, QSB], F32, tag=f"r1{h}")
                    nc.sync.dma_start(out=r1, in_=rsums[h:h + 1, :])
                    r1s[h] = r1
                # The per-head normalize (broadcast + multiply) is DEFERRED:
                # emitted between the next super-block's early gpairs so the
                # DVE queue interleaves it with new attention work instead of
                # head-of-line-blocking on the whole chain.
                otn = {}

                def make_norm(h, r1, ou, dst):
                    def emit():
                        rb = b_sm.tile([KC, QSB], F32, tag="rb")
                        nc.gpsimd.partition_broadcast(rb, r1, channels=KC)
                        on = b_ot.tile([HD, QSB], BF16, tag=f"otn{h}")
                        nc.vector.tensor_mul(on, ou, rb)
                        dst[h] = on
                    return emit

                norm_queue = [make_norm(h, r1s[h], otu[h], otn)
                              for h in HEAD_ORDER]
                if sb == NSB - 1:
                    for fn in norm_queue:
                        fn()
                    norm_queue = []

                otn_prev = otn
                prev_sb = sb

            # drain the last super-block's out-proj units
            for u in range(NU):
                emit_c_unit(prev_sb, u, otn_prev)


# ---------------------------------------------------------------------------
# Host-side input prep
# ---------------------------------------------------------------------------


def _rope_tables():
    inv_freq = 1.0 / (10000.0 ** (np.arange(0, HD, 2, dtype=np.float64) / HD))
    t = np.arange(T, dtype=np.float64)
    freqs = np.outer(t, inv_freq)                     # [T, 64]
    emb = np.concatenate([freqs, freqs], axis=-1)     # [T, 128]
    cosT = np.cos(emb).T.astype(np.float32)           # [128, T]
    sinT = np.sin(emb).T.astype(np.float32)
    # rotate_half(x) = [-x2, x1]; the device computes qrot = [x2, x1], so
    # fold the sign of the first half into the sin table.
    sinT[:HD // 2, :] *= -1.0
    return (np.ascontiguousarray(cosT.astype(BF16NP)),
            np.ascontiguousarray(sinT.astype(BF16NP)))


def _masks_t():
    # masks[r, j, c] = 1 if c >= j*128 + r  (causal mask for the diagonal
    # 512-wide block, per 128-key chunk j)
    r = np.arange(KC)[:, None, None]
    j = np.arange(QSB // KC)[None, :, None]
    c = np.arange(QSB)[None, None, :]
    return (c >= j * KC + r).astype(BF16NP)


def kernel(x, w_qkv, b_qkv, w_out, b_out):
    global LAST_EXEC_NS, LAST_RESULTS

    x = np.asarray(x, dtype=np.float32)
    w_qkv = np.asarray(w_qkv, dtype=np.float32)
    b_qkv = np.asarray(b_qkv, dtype=np.float32)
    w_out = np.asarray(w_out, dtype=np.float32)
    b_out = np.asarray(b_out, dtype=np.float32)

    if "prog" not in _PROGRAM_CACHE:
        _PROGRAM_CACHE["prog"] = _build_program()
    nc = _PROGRAM_CACHE["prog"]

    cosT, sinT = _rope_tables()
    masks = _masks_t()

    xTs = [np.ascontiguousarray(x[b].T.astype(BF16NP)) for b in range(B)]
    in_maps = []
    for c in range(N_CORES):
        b = c // G
        g = c % G
        f0 = g * HPG * HD
        f1 = (g + 1) * HPG * HD
        w_loc = np.ascontiguousarray(np.concatenate(
            [w_qkv[:, base + f0 + hp * 256: base + f0 + (hp + 1) * 256]
             for hp in range(HPG // 2)
             for base in (0, DIM, 2 * DIM)], axis=1).astype(BF16NP))
        b_loc = np.concatenate(
            [b_qkv[f0:f1], b_qkv[DIM + f0:DIM + f1],
             b_qkv[2 * DIM + f0:2 * DIM + f1]])
        b_cols = np.ascontiguousarray(
            b_loc.reshape(3 * HPG, HD).T).astype(np.float32)
        w_out_loc = np.ascontiguousarray(w_out[f0:f1, :].astype(BF16NP))
        in_maps.append({
            "xT": xTs[b],
            "w_qkv_loc": w_loc,
            "b_cols": b_cols,
            "w_out_loc": w_out_loc,
            "cosT": cosT,
            "sinTs": sinT,
            "masks_t": masks,
        })

    trace = bool(os.environ.get("BASS_KERNEL_TRACE"))
    res = run_bass_kernel_spmd(nc, in_maps, list(range(N_CORES)), trace=trace)
    LAST_EXEC_NS = res.exec_time_ns
    LAST_RESULTS = res

    out = np.empty((B, T, DIM), dtype=np.float32)
    for b in range(B):
        acc = res.results[4 * b]["y_part"].astype(np.float32)
        for g in range(1, G):
            acc = acc + res.results[4 * b + g]["y_part"].astype(np.float32)
        out[b] = acc + b_out[None, :]
    return out


# revision 32
# speedup vs baseline: 1.0430x; 1.0029x over previous
"""Causal self-attention (B=2, T=2048, dim=2048, 16 heads, RoPE) on 8 trn2
NeuronCores.

Sharding: core c handles batch b = c//4 and head group g = c%4 (4 heads each,
tensor-parallel over heads). Each core computes QKV projection + RoPE +
causal attention + its partial out-projection; the host sums the 4 partial
out-proj results per batch (the "all-reduce") and stacks batches.

v2 design (bf16 overhaul):
  - All matmuls in bf16: same PE streaming rate as float32r, but FWL halves
    the per-matmul LDWEIGHTS cost, DMA bytes halve, and DVE elementwise ops
    run at 2x on 16-bit.
  - Q^T/K^T/V stay SBUF-resident between projection and attention (no DRAM
    round trip).
  - RoPE rotate-half is a partition-shifted SBUF->SBUF DMA copy (the sign is
    folded into the host-built sin table), not a PE matmul.
  - V is transposed [d,t]->[t,d] with the DMA XBAR transpose, not PE.
  - Softmax denominators for all 4 heads of a query super-block accumulate
    into one [4, 512] PSUM tile via per-head one-hot ones columns, so one
    [4,512] reciprocal replaces 16 broadcast [128,512] reciprocals.
  - Phases B (attention) and C (out-proj) are merged, super-block-outer:
    each 512-query block's out-projection runs as soon as its softmax is
    normalized, overlapping y DMA writes with later attention.
  - QKV bias is applied on the Scalar engine during PSUM evacuation;
    the output bias is added on the host after the partial sum.
"""

import math
import os
import sys
import types

import numpy as np
import ml_dtypes

BF16NP = ml_dtypes.bfloat16

# ---------------------------------------------------------------------------
# NTFF profile hook (missing antenv.axon_hooks in this image). Reconstructed
# so run_bass_kernel_spmd(trace=True) can measure HW exec time.
# ---------------------------------------------------------------------------
try:
    import antenv

    if "antenv.axon_hooks" not in sys.modules:
        try:
            from trn_agent_boot.trn_boot import _ntff_profile_via_ctypes

            _hook = _ntff_profile_via_ctypes("/opt/axon/libaxon_pjrt.so")
        except Exception:
            _hook = None
        _m = types.ModuleType("antenv.axon_hooks")
        _m.get_axon_ntff_profile_hook = lambda: _hook
        _m.set_axon_ntff_profile_hook = lambda h: None
        sys.modules["antenv.axon_hooks"] = _m
        antenv.axon_hooks = _m
except Exception:
    pass

import concourse.bass as bass
import concourse.tile as tile
from concourse import bacc, mybir
from concourse.bass_utils import run_bass_kernel_spmd

# Problem constants (hardcoded per the task contract).
B = 2
T = 2048
DIM = 2048
H = 16
HD = 128                  # head_dim
G = 4                     # head groups (cores per batch)
HPG = H // G              # heads per group = 4
N_CORES = 8
SCALE = 1.0 / math.sqrt(HD)

F32 = mybir.dt.float32
BF16 = mybir.dt.bfloat16

TSL = 512                 # t-slice width in the projection phase
NTSL = T // TSL           # 4
QSB = 512                 # query super-block width in the attention phase
NSB = T // QSB            # 4
KC = 128                  # key chunk (partition dim)

LAST_EXEC_NS = None
LAST_RESULTS = None

_PROGRAM_CACHE = {}


def _build_program():
    nc = bacc.Bacc("TRN2", target_bir_lowering=False, debug=False,
                   num_devices=N_CORES)

    xT = nc.dram_tensor("xT", [DIM, T], BF16, kind="ExternalInput").ap()
    w_qkv = nc.dram_tensor("w_qkv_loc", [DIM, 3 * HPG * HD], BF16,
                           kind="ExternalInput").ap()
    b_cols = nc.dram_tensor("b_cols", [HD, 3 * HPG], F32,
                            kind="ExternalInput").ap()
    w_out = nc.dram_tensor("w_out_loc", [HPG * HD, DIM], BF16,
                           kind="ExternalInput").ap()
    cosT = nc.dram_tensor("cosT", [HD, T], BF16, kind="ExternalInput").ap()
    sinT = nc.dram_tensor("sinTs", [HD, T], BF16, kind="ExternalInput").ap()
    masks = nc.dram_tensor("masks_t", [KC, QSB // KC, QSB], BF16,
                           kind="ExternalInput").ap()
    y = nc.dram_tensor("y_part", [T, DIM], BF16, kind="ExternalOutput").ap()

    with tile.TileContext(nc) as tc:
        _emit(tc, nc, xT, w_qkv, b_cols, w_out, cosT, sinT, masks, y)

    nc.compile()
    return nc


def _emit(tc, nc, xT, w_qkv, b_cols_d, w_out, cosT_d, sinT_d, masks_d, y):
    from contextlib import ExitStack

    ctx = ExitStack()
    with ctx:
        ctx.enter_context(nc.allow_low_precision(
            reason="bf16 matmul operands and elementwise pipeline"))
        # ---------------- constants (live for the whole kernel) -----------
        consts = ctx.enter_context(tc.tile_pool(name="consts", bufs=1))
        bcols = consts.tile([HD, 3 * HPG], F32, tag="bcols")
        nc.gpsimd.dma_start(out=bcols, in_=b_cols_d)
        # ones4[:, h, :] is the [128, 4] one-hot stationary for head h: only
        # column h is ones, so head h's softmax-sum matmul lands in row h of
        # the shared [HPG, QSB] PSUM accumulator (other rows accumulate +0).
        ones4 = consts.tile([KC, HPG, HPG], BF16, tag="ones4")
        nc.vector.memset(ones4, 0.0)
        for h in range(HPG):
            nc.vector.memset(ones4[:, h, h:h + 1], 1.0)
        masks_sb = consts.tile([KC, QSB // KC, QSB], BF16, tag="masks")
        nc.gpsimd.dma_start(out=masks_sb, in_=masks_d)

        # QKV, attention output: SBUF-resident for the whole kernel.
        qkv_pool = ctx.enter_context(tc.tile_pool(name="qkv", bufs=1))
        qtr = [qkv_pool.tile([HD, T], BF16, tag=f"qtr{h}", name=f"qtr{h}")
               for h in range(HPG)]
        ktr = [qkv_pool.tile([HD, T], BF16, tag=f"ktr{h}", name=f"ktr{h}")
               for h in range(HPG)]
        vh = [qkv_pool.tile([KC, T // KC, HD], BF16, tag=f"vh{h}",
                            name=f"vh{h}")
              for h in range(HPG)]

        rope = ctx.enter_context(tc.tile_pool(name="rope", bufs=1))
        cosT = rope.tile([HD, T], BF16, tag="cosT")
        sinT = rope.tile([HD, T], BF16, tag="sinT")

        xT_r = xT.rearrange("(c p) t -> p c t", p=KC)        # [128, 16, T]
        w_r = w_qkv.rearrange("(c p) f -> p c f", p=KC)      # [128, 16, 1536]
        NKCH = DIM // KC                                     # 16 k-chunks

        w_out_r = w_out.rearrange("(c p) o -> p c o", p=KC)

        # ======================= Phase A: QKV + RoPE ======================
        with (
            tc.tile_pool(name="a_w", bufs=1) as a_w,
            tc.tile_pool(name="a_x", bufs=4) as a_x,
            tc.tile_pool(name="a_vb", bufs=3) as a_vb,
            tc.tile_pool(name="a_qb", bufs=5) as a_qb,
            tc.tile_pool(name="a_rot", bufs=5) as a_rot,
            tc.tile_pool(name="a_m1", bufs=5) as a_m1,
            tc.tile_pool(name="a_ps", bufs=8, space="PSUM") as a_ps,
        ):
            # Two HWDGE queues: SP (nc.sync) carries the latency-sensitive
            # small DMAs (rope rotations, V transposes) that are emitted
            # throughout phase A; the WAR-free bulk preloads go on the
            # Activation queue (nc.scalar) so they never head-of-line-block
            # them. xsl3 reuses xsl0's buffer (a WAR wait), so it is emitted
            # late on SP, where the wait is met by the time it reaches the
            # queue head.
            def load_xsl(tsl, q):
                t0 = tsl * TSL
                xs = a_x.tile([KC, NKCH, TSL], BF16, tag="xsl",
                              name=f"xsl{tsl}")
                for jj in range(4):
                    q.dma_start(
                        out=xs[:, jj * 4:(jj + 1) * 4, :],
                        in_=xT_r[:, jj * 4:(jj + 1) * 4, t0:t0 + TSL])
                return xs

            xsls = [load_xsl(0, nc.sync)]
            # Bulk preloads are split across BOTH spare queues so no single
            # per-queue DMA-completion counter spans the whole ~16MB (a
            # coarse semaphore wait on "bulk queue drained" stalled the
            # evac pipeline until the last byte landed): x slices ride the
            # Activation queue (descriptor gen finishes before the first
            # evac needs it), weights/rope/masks ride GpSimd, rope tables
            # first. x is fully resident (4 buffers): no WAR loads.
            nc.gpsimd.dma_start(out=cosT, in_=cosT_d)
            nc.gpsimd.dma_start(out=sinT, in_=sinT_d)
            xsls.append(load_xsl(1, nc.scalar))
            # w_qkv_loc is host-packed head-pair-major:
            # [hp0: q(2 heads), k, v | hp1: q, k, v], 256 cols per block.
            # Full 1536-col rows per k-chunk: both head-pairs' kc0 weights
            # arrive after the first DMA, so neither hp ever waits.
            w_all = a_w.tile([KC, NKCH, 3 * HPG * HD], BF16, tag="w_all")
            for kc in range(NKCH):
                nc.gpsimd.dma_start(out=w_all[:, kc, :],
                                    in_=w_r[:, kc, :])
            xsls.append(load_xsl(2, nc.scalar))
            xsls.append(load_xsl(3, nc.scalar))

            for tsl in range(NTSL):
                t0 = tsl * TSL
                xsl = xsls[tsl]

                # Last t-slice: process heads (2,3) first so the final psum
                # evac chain belongs to heads (0,1); phase B starts with head
                # 2, whose data (and psum banks) free up first.
                hp_order = (1, 0) if tsl == NTSL - 1 else (0, 1)
                for hp in hp_order:
                    heads = (2 * hp, 2 * hp + 1)
                    outs = [(h, kind) for h in heads for kind in range(3)]
                    # kc-outer: six psum accumulators advance together, so the
                    # PE tracks weight-chunk DMA arrival instead of stalling on
                    # the full weight load.
                    pstiles = {}
                    for (h, kind) in outs:
                        pstiles[(h, kind)] = a_ps.tile(
                            [HD, TSL], F32, tag="ps_qkv",
                            name=f"ps_{tsl}_{h}_{kind}")
                    for kc in range(NKCH):
                        for (h, kind) in outs:
                            feat0 = (h // 2) * 768 + kind * 256 + (h % 2) * HD
                            nc.tensor.matmul(
                                pstiles[(h, kind)],
                                w_all[:, kc, feat0:feat0 + HD],
                                xsl[:, kc, :],
                                start=(kc == 0), stop=(kc == NKCH - 1),
                            )
                    # Staged evacuation: each stage fans out across its
                    # engine queue before the next depends on it, so no
                    # queue head-of-line-blocks another (the serial
                    # per-chain version stalled phase-B's PSUM reuse).
                    # 1) V evacs (DVE) -> 2) q/k evacs (Scalar) ->
                    # 3) V transposes -> 4) rotation DMAs ->
                    # 5) m1 muls (overlap the rotations in flight) ->
                    # 6) m2 muls -> 7) adds into qtr/ktr.
                    vbs, qbs, qrots, m1s = {}, {}, {}, {}
                    for h in heads:
                        vb = a_vb.tile([HD, TSL], BF16, tag="vb")
                        nc.vector.tensor_scalar_add(
                            vb, pstiles[(h, 2)],
                            bcols[:, 2 * HPG + h:2 * HPG + h + 1])
                        vbs[h] = vb
                    for h in heads:
                        for kind in (0, 1):
                            qb = a_qb.tile([HD, TSL], F32, tag="qb")
                            nc.scalar.activation(
                                qb, pstiles[(h, kind)],
                                mybir.ActivationFunctionType.Identity,
                                bias=bcols[:, kind * HPG + h:
                                           kind * HPG + h + 1])
                            qbs[(h, kind)] = qb
                    for h in heads:
                        nc.sync.dma_start_transpose(
                            out=vh[h][:, tsl * (TSL // KC):
                                      (tsl + 1) * (TSL // KC), :],
                            in_=vbs[h])
                    half = HD // 2
                    for h in heads:
                        for kind in (0, 1):
                            qb = qbs[(h, kind)]
                            # rotate-half: partition-shifted SBUF->SBUF copy
                            # (sign folded into the host-built sin table)
                            qrot = a_rot.tile([HD, TSL], F32, tag="qrot")
                            nc.sync.dma_start(out=qrot[0:half, :],
                                              in_=qb[half:HD, :])
                            nc.sync.dma_start(out=qrot[half:HD, :],
                                              in_=qb[0:half, :])
                            qrots[(h, kind)] = qrot
                    for h in heads:
                        for kind in (0, 1):
                            m1 = a_m1.tile([HD, TSL], F32, tag="m1")
                            nc.vector.tensor_mul(m1, qbs[(h, kind)],
                                                 cosT[:, t0:t0 + TSL])
                            m1s[(h, kind)] = m1
                    for h in heads:
                        for kind in (0, 1):
                            # m2 in place on qrot (saves an SBUF ring)
                            qrot = qrots[(h, kind)]
                            nc.vector.tensor_mul(qrot, qrot,
                                                 sinT[:, t0:t0 + TSL])
                            dst = qtr[h] if kind == 0 else ktr[h]
                            nc.vector.tensor_add(dst[:, t0:t0 + TSL],
                                                 m1s[(h, kind)], qrot)



        # ================= Phase B+C: attention + out-proj ================
        # Software-pipelined emission (the Tensor queue executes in order, so
        # emission order IS the PE schedule):
        #   - head order [2,3,0,1] matches the phase-A evac order above;
        #   - softmax-sum matmuls are emitted one gpair late, so the next
        #     super-block's first sum matmul never head-of-line-blocks on the
        #     previous block's reciprocal (same PSUM bank);
        #   - each super-block's out-proj units are interleaved into the NEXT
        #     super-block's attention gpairs (starting at gpair 4, giving the
        #     normalize chain time to finish); the last block's units drain at
        #     the end.
        HEAD_ORDER = (2, 3, 0, 1)
        NOB = DIM // 512
        NU = (QSB // KC) * NOB                    # 16 out-proj units per sb
        # out-proj weights land in the space freed by phase A; the first
        # consumer (an out-proj unit of sb0) runs ~25us into phase B.
        c_w = ctx.enter_context(tc.tile_pool(name="c_w", bufs=1))
        wo = c_w.tile([KC, HPG, DIM], BF16, tag="wo")
        for hc in range(HPG):
            nc.gpsimd.dma_start(out=wo[:, hc, :], in_=w_out_r[:, hc, :])
        with (
            tc.tile_pool(name="b_pt", bufs=4) as b_pt,
            tc.tile_pool(name="b_ot", bufs=2) as b_ot,
            tc.tile_pool(name="b_sm", bufs=2) as b_sm,
            tc.tile_pool(name="c_sb", bufs=4) as c_sb,
            tc.tile_pool(name="b_ps_s", bufs=2, space="PSUM") as b_ps_s,
            tc.tile_pool(name="b_ps_o", bufs=1, space="PSUM") as b_ps_o,
            tc.tile_pool(name="b_ps_sum", bufs=1, space="PSUM") as b_ps_sum,
            tc.tile_pool(name="c_ps", bufs=2, space="PSUM") as c_ps,
        ):
            def emit_c_unit(csb, u, otn_map):
                tb, ob = divmod(u, NOB)
                tt0 = tb * KC
                o0 = ob * 512
                ps_y = c_ps.tile([KC, 512], F32, tag="ps_y")
                # accumulate in HEAD_ORDER: the first otn to be normalized is
                # needed first, shrinking the drain stall after the last sb
                for i, hc in enumerate(HEAD_ORDER):
                    nc.tensor.matmul(
                        ps_y, otn_map[hc][:, tt0:tt0 + KC],
                        wo[:, hc, o0:o0 + 512],
                        start=(i == 0), stop=(i == HPG - 1),
                    )
                ys = c_sb.tile([KC, 512], BF16, tag="ys")
                if u % 2 == 0:
                    nc.scalar.activation(
                        ys, ps_y, mybir.ActivationFunctionType.Identity)
                else:
                    nc.vector.tensor_copy(ys, ps_y)
                r0 = csb * QSB + tt0
                nc.sync.dma_start(out=y[r0:r0 + KC, o0:o0 + 512], in_=ys)

            otn_prev = None
            prev_sb = None
            norm_queue = []
            for sb in range(NSB):
                q0 = sb * QSB
                nk = (sb + 1) * (QSB // KC)       # causal key chunks
                ngp = nk // 2
                total_gp = HPG * ngp
                # schedule prev sb's NU out-proj units over gpair slots >= 8
                # (by slot 8 the previous block's normalize chain is done)
                slot_units = {}
                if otn_prev is not None:
                    lo = 8 if total_gp > 9 else total_gp - 1
                    span = max(total_gp - lo, 1)
                    for u in range(NU):
                        s = lo + (u * span) // NU
                        slot_units.setdefault(min(s, total_gp - 1),
                                              []).append(u)

                ps_sum = b_ps_sum.tile([HPG, QSB], F32, tag="ps_sum")
                pending_norms = list(norm_queue)
                norm_queue = []
                sum_queue = []      # sums run two gpairs late: the previous
                sum_first = True    # reciprocal is long done by then

                def flush_sum(last):
                    nonlocal sum_first
                    while sum_queue and (len(sum_queue) > 2 or last):
                        fh, fpt, fc0s = sum_queue.pop(0)
                        for j in range(2):
                            nc.tensor.matmul(
                                ps_sum[:, fc0s[j]:], ones4[:, fh, :],
                                fpt[:, j, fc0s[j]:],
                                start=(sum_first and j == 0),
                                stop=(last and not sum_queue and j == 1),
                            )
                        sum_first = False

                otu = {}
                gslot = 0
                for h in HEAD_ORDER:
                    ps_o = b_ps_o.tile([HD, QSB], F32, tag="ps_o")
                    for gpair in range(ngp):
                        k0 = 2 * gpair
                        # Diagonal chunks (dj >= 0) only attend to queries
                        # q >= dj*128: trim the streamed column range of the
                        # S/O/sum matmuls, exp, and mask to the valid part.
                        # (The trimmed-off region of pt/psum is stale but is
                        # never read.)
                        djs = [(k0 + j) - (nk - QSB // KC) for j in range(2)]
                        c0s = [max(dj, 0) * KC for dj in djs]
                        cmin = min(c0s)
                        ps_st = b_ps_s.tile([KC, 2, QSB], F32, tag="ps_st")
                        for j in range(2):
                            c0 = c0s[j]
                            nc.tensor.matmul(
                                ps_st[:, j, c0:],
                                ktr[h][:, (k0 + j) * KC:(k0 + j + 1) * KC],
                                qtr[h][:, q0 + c0:q0 + QSB],
                                start=True, stop=True,
                            )
                        for u in slot_units.get(gslot, ()):
                            emit_c_unit(prev_sb, u, otn_prev)
                        pt = b_pt.tile([KC, 2, QSB], BF16, tag="pt")
                        nc.scalar.activation(
                            pt[:, :, cmin:], ps_st[:, :, cmin:],
                            mybir.ActivationFunctionType.Exp, scale=SCALE)
                        for j in range(2):
                            dj = djs[j]
                            if dj >= 0:
                                c0 = c0s[j]
                                nc.vector.tensor_mul(
                                    pt[:, j, c0:], pt[:, j, c0:],
                                    masks_sb[:, dj, c0:])
                        for j in range(2):
                            kci = k0 + j
                            nc.tensor.matmul(
                                ps_o[:, c0s[j]:], vh[h][:, kci, :],
                                pt[:, j, c0s[j]:],
                                start=(kci == 0), stop=(kci == nk - 1),
                            )
                        sum_queue.append((h, pt, c0s))
                        flush_sum(False)
                        gslot += 1
                        if pending_norms and gslot >= 1:
                            pending_norms.pop(0)()
                    # evacuate unnormalized O^T (bf16); normalized after the
                    # batched reciprocal below.
                    ou = b_ot.tile([HD, QSB], BF16, tag=f"otu{h}")
                    nc.vector.tensor_copy(ou, ps_o)
                    otu[h] = ou
                flush_sum(True)

                # batched reciprocal: one [4, 512] op for all heads
                rsums = b_sm.tile([HPG, QSB], F32, tag="rsums")
                nc.vector.reciprocal(rsums, ps_sum)
                # partition_broadcast requires its input at partition 0:
                # stage all 4 rows there first (the DMAs run concurrently).
                r1s = {}
                for h in HEAD_ORDER:
                    r1 = b_sm.tile([1, QSB], F32, tag=f"r1{h}")
                    nc.sync.dma_start(out=r1, in_=rsums[h:h + 1, :])
                    r1s[h] = r1
                # The per-head normalize (broadcast + multiply) is DEFERRED:
                # emitted between the next super-block's early gpairs so the
                # DVE queue interleaves it with new attention work instead of
                # head-of-line-blocking on the whole chain.
                otn = {}

                def make_norm(h, r1, ou, dst):
                    def emit():
                        rb = b_sm.tile([KC, QSB], F32, tag="rb")
                        nc.gpsimd.partition_broadcast(rb, r1, channels=KC)
                        on = b_ot.tile([HD, QSB], BF16, tag=f"otn{h}")
                        nc.vector.tensor_mul(on, ou, rb)
                        dst[h] = on
                    return emit

                norm_queue = [make_norm(h, r1s[h], otu[h], otn)
                              for h in HEAD_ORDER]
                if sb == NSB - 1:
                    for fn in norm_queue:
                        fn()
                    norm_queue = []

                otn_prev = otn
                prev_sb = sb

            # drain the last super-block's out-proj units
            for u in range(NU):
                emit_c_unit(prev_sb, u, otn_prev)


# ---------------------------------------------------------------------------
# Host-side input prep
# ---------------------------------------------------------------------------


def _rope_tables():
    inv_freq = 1.0 / (10000.0 ** (np.arange(0, HD, 2, dtype=np.float64) / HD))
    t = np.arange(T, dtype=np.float64)
    freqs = np.outer(t, inv_freq)                     # [T, 64]
    emb = np.concatenate([freqs, freqs], axis=-1)     # [T, 128]
    cosT = np.cos(emb).T.astype(np.float32)           # [128, T]
    sinT = np.sin(emb).T.astype(np.float32)
    # rotate_half(x) = [-x2, x1]; the device computes qrot = [x2, x1], so
    # fold the sign of the first half into the sin table.
    sinT[:HD // 2, :] *= -1.0
    return (np.ascontiguousarray(cosT.astype(BF16NP)),
            np.ascontiguousarray(sinT.astype(BF16NP)))


def _masks_t():
    # masks[r, j, c] = 1 if c >= j*128 + r  (causal mask for the diagonal
    # 512-wide block, per 128-key chunk j)
    r = np.arange(KC)[:, None, None]
    j = np.arange(QSB // KC)[None, :, None]
    c = np.arange(QSB)[None, None, :]
    return (c >= j * KC + r).astype(BF16NP)


def kernel(x, w_qkv, b_qkv, w_out, b_out):
    global LAST_EXEC_NS, LAST_RESULTS

    x = np.asarray(x, dtype=np.float32)
    w_qkv = np.asarray(w_qkv, dtype=np.float32)
    b_qkv = np.asarray(b_qkv, dtype=np.float32)
    w_out = np.asarray(w_out, dtype=np.float32)
    b_out = np.asarray(b_out, dtype=np.float32)

    if "prog" not in _PROGRAM_CACHE:
        _PROGRAM_CACHE["prog"] = _build_program()
    nc = _PROGRAM_CACHE["prog"]

    cosT, sinT = _rope_tables()
    masks = _masks_t()

    xTs = [np.ascontiguousarray(x[b].T.astype(BF16NP)) for b in range(B)]
    in_maps = []
    for c in range(N_CORES):
        b = c // G
        g = c % G
        f0 = g * HPG * HD
        f1 = (g + 1) * HPG * HD
        w_loc = np.ascontiguousarray(np.concatenate(
            [w_qkv[:, base + f0 + hp * 256: base + f0 + (hp + 1) * 256]
             for hp in range(HPG // 2)
             for base in (0, DIM, 2 * DIM)], axis=1).astype(BF16NP))
        b_loc = np.concatenate(
            [b_qkv[f0:f1], b_qkv[DIM + f0:DIM + f1],
             b_qkv[2 * DIM + f0:2 * DIM + f1]])
        b_cols = np.ascontiguousarray(
            b_loc.reshape(3 * HPG, HD).T).astype(np.float32)
        w_out_loc = np.ascontiguousarray(w_out[f0:f1, :].astype(BF16NP))
        in_maps.append({
            "xT": xTs[b],
            "w_qkv_loc": w_loc,
            "b_cols": b_cols,
            "w_out_loc": w_out_loc,
            "cosT": cosT,
            "sinTs": sinT,
            "masks_t": masks,
        })

    trace = bool(os.environ.get("BASS_KERNEL_TRACE"))
    res = run_bass_kernel_spmd(nc, in_maps, list(range(N_CORES)), trace=trace)
    LAST_EXEC_NS = res.exec_time_ns
    LAST_RESULTS = res

    out = np.empty((B, T, DIM), dtype=np.float32)
    for b in range(B):
        acc = res.results[4 * b]["y_part"].astype(np.float32)
        for g in range(1, G):
            acc = acc + res.results[4 * b + g]["y_part"].astype(np.float32)
        out[b] = acc + b_out[None, :]
    return out
